# revision 1
# baseline (speedup 1.0000x reference)
"""
BinaryTreeShConv Trainium2 kernel (8-core SPMD, data-parallel over batch B=8).

Reference computation per (b, v):
    patches[p, c]   = signal[b, idx[b,v,p], c]                      (gather, P=32, C=32)
    Y1[c, rn]       = sum_p conv[b,v,p,rn] * patches[p, c]          (RN = R*N = 32)
    out[b, v, i]    = relu(bias[i] + sum_{c,rn} W[i,c,rn] * Y1[c,rn])

Device mapping (per core = one batch b):
  - Vertices processed in groups of 4 (vv in 0..3) packed along the matmul
    contraction dim K = (vv, p) = 128.
  - Patch rows are gathered from a host-padded bf16 signal (rows padded to
    256 B) via gpsimd dma_gather: gathered row i lands on partition i%128,
    slot i//128 -> a dense [128=(vv,p), G, 128] bf16 tile per v-tile; the
    matmuls read only the first 32 columns of each slot.
  - Step 2 matmul per group g: lhsT (stationary) = block-diagonal conv
    [128=(vv,p), 128=(vv,rn)] bf16 (built on-chip by 4 small DVE copies per
    tile into a once-memset buffer; bf16 128-col loads get fast weight load),
    rhs (moving) = gathered patches [128, 32]; out = [(vv,rn), c] in PSUM,
    16 groups per PSUM bank.
  - Step 3: out[i, v] accumulated over c in PSUM: for each c a matmul with
    lhsT = W[:, c, :]^T [rn, i] fp32 (stationary, replicated in the 4 PE
    row-groups = vv blocks) against rhs = Y1[rn, v-slot] fp32 slices.
  - bias + relu fused into one ACT instruction per sweep (PSUM -> SBUF).
  - Host pre-arranges conv / idx / W layouts and un-permutes the output
    column order (host-side numpy; all value-dependent work is on-device).
"""

import os
import numpy as np
import ml_dtypes

from concourse import bacc, bass, mybir
import concourse.tile as tile
from concourse import bass_utils
from concourse.bass import ds, ts

# Problem constants (hardcoded; kernel.py must be self-contained).
B = 8
V = 4096
P = 32
C = 32
R = 2
N_SH = 16
RN = R * N_SH  # 32
OUT = 32

NG = V // 4          # 1024 groups of 4 vertices
G = 64               # groups per tile
NT = NG // G         # 16 tiles
NIDX = 128 * G       # gathered rows per tile (8192)
U = 2 * G            # u-slots per step-3 sweep (2 tiles)
SIGW = 128           # padded signal row, bf16 elements (256 B)

_f32 = mybir.dt.float32
_bf16 = mybir.dt.bfloat16
_i16 = mybir.dt.int16

_NC = None
LAST_RESULTS = None
_LAST_IN_MAPS = None


def _build_program():
    nc = bacc.Bacc("TRN2", target_bir_lowering=False, debug=False)

    conv_d = nc.dram_tensor("conv", (128, NG * RN), _bf16, kind="ExternalInput")
    sigp_d = nc.dram_tensor("sigp", (V, SIGW), _bf16, kind="ExternalInput")
    idx_d = nc.dram_tensor("idx", (128, NT * (NIDX // 16)), _i16,
                           kind="ExternalInput")
    w_d = nc.dram_tensor("w", (128, C * OUT), _f32, kind="ExternalInput")
    bias_d = nc.dram_tensor("bias", (OUT, 1), _f32, kind="ExternalInput")
    out_d = nc.dram_tensor("out", (OUT, V), _f32, kind="ExternalOutput")

    with tile.TileContext(nc) as tc:
        _kernel_body(tc, conv_d.ap(), sigp_d.ap(), idx_d.ap(), w_d.ap(),
                     bias_d.ap(), out_d.ap())

    nc.compile()
    return nc


def _kernel_body(tc, conv_d, sigp_d, idx_d, w_d, bias_d, out_d):
    nc = tc.nc
    IW = NIDX // 16  # idx words per tile per partition (512)

    with tc.tile_pool(name="sb", bufs=1) as sb, \
         tc.tile_pool(name="ps2", bufs=4, space="PSUM") as pp2, \
         tc.tile_pool(name="ps3", bufs=1, space="PSUM") as pp3:

        w_t = sb.tile([128, C * OUT], _f32, tag="w")
        bias_t = sb.tile([OUT, 1], _f32, tag="bias")
        out_sb = sb.tile([OUT, V], _f32, tag="out_sb")
        patches = [sb.tile([128, G, SIGW], _bf16, tag=f"patch{s}",
                           name=f"patch{s}") for s in range(2)]
        convt = [sb.tile([128, G, RN], _bf16, tag=f"conv{s}",
                         name=f"convt{s}") for s in range(2)]
        convbd = [sb.tile([128, G, 128], _bf16, tag=f"convbd{s}",
                          name=f"convbd{s}") for s in range(2)]
        idxt = [sb.tile([128, IW], _i16, tag=f"idx{s}", name=f"idxt{s}")
                for s in range(2)]
        y1 = [sb.tile([128, C, U], _f32, tag=f"y1{s}", name=f"y1_{s}")
              for s in range(2)]

        nc.sync.dma_start(w_t[:], w_d[:])
        nc.sync.dma_start(bias_t[:], bias_d[:])
        # Off-block-diagonal zeros, written once; the per-tile copies only
        # overwrite the diagonal blocks.
        nc.vector.memset(convbd[0][:], 0.0)
        nc.vector.memset(convbd[1][:], 0.0)

        for t in range(NT):
            s = t % 2
            parity = t % 2
            sweep = t // 2
            s_y = sweep % 2  # a sweep spans two tiles; both write this buffer

            nc.sync.dma_start(idxt[s][:], idx_d[:, ts(t, IW)])
            nc.gpsimd.dma_gather(
                out_ap=patches[s][:],
                in_ap=sigp_d[:],
                idxs_ap=idxt[s][:],
                num_idxs=NIDX,
                num_idxs_reg=NIDX,
                elem_size=SIGW,
                # >64 descriptors per SDMA engine do not fit one packet
                single_packet=False,
            )
            nc.sync.dma_start(convt[s][:],
                              conv_d[:, ds(t * G * RN, G * RN)]
                              .rearrange("k (g r) -> k g r", r=RN))
            for vv in range(4):
                nc.vector.tensor_copy(
                    out=convbd[s][32 * vv:32 * vv + 32, :,
                                  32 * vv:32 * vv + 32],
                    in_=convt[s][32 * vv:32 * vv + 32, :, :],
                )

            # --- step 2: one matmul per group, 16 groups per PSUM bank ---
            for q in range(G // 16):
                ps = pp2.tile([128, 16, C], _f32, tag="ps2",
                              name=f"ps2_{t}_{q}")
                for j in range(16):
                    g = q * 16 + j
                    nc.tensor.matmul(
                        out=ps[:, j],
                        lhsT=convbd[s][:, g, :],
                        rhs=patches[s][:, g, 0:C],
                        # one start per bank: start zeroes the whole 2KB
                        # zero-region; later groups overwrite-on-pending.
                        start=(j == 0), stop=(j == 15),
                        skip_group_check=True,
                    )
                # ps[(vv,rn), j, c] -> y1[(vv,rn), c, u], u = parity*G+q*16+j
                dst = y1[s_y][:, :, ds(parity * G + q * 16, 16)]
                nc.vector.tensor_copy(out=dst.rearrange("p c j -> p j c"),
                                      in_=ps[:])

            # --- step 3 sweep over two tiles' worth of Y1 ---
            # Row-tiled matmuls (distinct PE row-groups) must drain into
            # DISTINCT PSUM banks -- same-bank row-tiling wedges the device.
            if parity == 1:
                psO = [pp3.tile([OUT, U], _f32, tag=f"ps3_{vv}",
                                name=f"ps3_{sweep}_{vv}") for vv in range(4)]
                for c in range(C):
                    for vv in range(4):
                        nc.tensor.matmul(
                            out=psO[vv][:],
                            lhsT=w_t[32 * vv:32 * vv + 32, ds(c * OUT, OUT)],
                            rhs=y1[s_y][32 * vv:32 * vv + 32, c, :],
                            start=(c == 0), stop=(c == C - 1),
                            tile_position=(32 * vv, 0),
                        )
                for vv in range(4):
                    nc.scalar.activation(
                        out=out_sb[:, ds(sweep * 4 * U + vv * U, U)],
                        in_=psO[vv][:],
                        func=mybir.ActivationFunctionType.Relu,
                        bias=bias_t[:],
                    )

        nc.sync.dma_start(out_d[:], out_sb[:])


def _host_arrange(conv_b, idx_b):
    # conv_b: [V, P, RN] f32 -> bf16 [128, NG*RN]: row vv*32+p, col gg*32+rn
    c = conv_b.reshape(NG, 4, P, RN)                    # [gg, vv, p, rn]
    c = np.ascontiguousarray(c.transpose(1, 2, 0, 3))   # [vv, p, gg, rn]
    conv_arr = c.reshape(128, NG * RN).astype(ml_dtypes.bfloat16)
    # idx_b: [V, P] int -> wrapped-16 int16 [128, NT*(NIDX//16)]
    # flat order per tile: i = g*128 + vv*32 + p  (natural ravel of [g,vv,p])
    flat = idx_b.reshape(NT, NIDX).astype(np.int16)
    wrapped = flat.reshape(NT, NIDX // 16, 16).transpose(0, 2, 1)  # [t,16,IW]
    wrapped = np.concatenate(list(wrapped), axis=1)     # [16, NT*IW]
    idx_arr = np.ascontiguousarray(np.tile(wrapped, (8, 1)))  # [128, NT*IW]
    return conv_arr, idx_arr


def _out_perm():
    # column j of device out -> vertex v
    j = np.arange(V)
    s2, r = j // (4 * U), j % (4 * U)
    vv, u = r // U, r % U
    parity, ru = u // G, u % G
    v = (2 * s2 + parity) * (4 * G) + ru * 4 + vv
    return v


def kernel(signal, patches_idx, conv_kernel, kernel_weights, biases):
    global _NC, LAST_RESULTS, _LAST_IN_MAPS

    signal = np.asarray(signal, dtype=np.float32)
    patches_idx = np.asarray(patches_idx)
    conv_kernel = np.asarray(conv_kernel, dtype=np.float32)
    kernel_weights = np.asarray(kernel_weights, dtype=np.float32)
    biases = np.asarray(biases, dtype=np.float32)

    if _NC is None:
        _NC = _build_program()
    nc = _NC

    # W: [OUT, C, R, N] -> w_arr[vv*32+rn, c*32+i] = W[i, c, rn], 4 replicas
    w3 = kernel_weights.reshape(OUT, C, RN)
    w_arr = np.ascontiguousarray(w3.transpose(2, 1, 0)).reshape(RN, C * OUT)
    w_arr = np.tile(w_arr, (4, 1)).astype(np.float32)
    bias_arr = np.ascontiguousarray(biases.reshape(OUT, 1))

    in_maps = []
    for b in range(B):
        conv_arr, idx_arr = _host_arrange(
            conv_kernel[b].reshape(V, P, RN), patches_idx[b])
        sigp = np.zeros((V, SIGW), dtype=ml_dtypes.bfloat16)
        sigp[:, :C] = signal[b].astype(ml_dtypes.bfloat16)
        in_maps.append({
            "conv": conv_arr,
            "sigp": sigp,
            "idx": idx_arr,
            "w": w_arr,
            "bias": bias_arr,
        })

    _LAST_IN_MAPS = in_maps
    trace = bool(int(os.environ.get("KERNEL_TRACE", "0")))
    res = bass_utils.run_bass_kernel_spmd(
        nc, in_maps, core_ids=list(range(B)), trace=trace,
    )
    LAST_RESULTS = res

    perm = _out_perm()
    out = np.empty((B, V, OUT), dtype=np.float32)
    for b in range(B):
        dev = res.results[b]["out"]          # [OUT, V] in device column order
        out[b, perm, :] = dev.T
    return out

